# revision 21
# baseline (speedup 1.0000x reference)
"""Trainium2 Bass kernel: MemoryGCNConv (GCN conv + memory routing + BN + L2 norm).

Strategy (8 NeuronCores, SPMD):
  - 392 destination tiles of 128 nodes are snake-dealt onto (core, slot)
    pairs by descending edge-chunk count, so every core runs the SAME
    per-slot chunk schedule (KEc[s] edge chunks + KMc[s] memory chunks,
    sized to the max over the 8 tiles sharing the slot).  This keeps one
    compiled program for all cores, near-exact lane counts, and zero
    cross-core skew.
  - Each slot gathers its per-edge source rows from a replicated fp16
    copy of x with dma_gather (signed int16 indices, table base at row
    IBASE so all 50176 rows are addressable); gather windows round-robin
    over 4 SWDGE queues so Q7 descriptor generation runs in parallel.
  - One-hot matrices for the tensor-engine segment-sum are generated one
    TILE at a time with broadcast-AP tensor_tensor ops (is_equal + mult)
    instead of per-chunk tensor_scalar ops.
  - Self-loops never ride the gather: each core bulk-loads its own x
    rows (slot order) and adds dinv^2 * x directly.
  - Memory messages ride the same gather windows, get W_mem + Lrelu
    (scalar-engine activation) per chunk, and aggregate with a wide
    one-hot.
  - BatchNorm statistics are reduced across cores with an AllReduce of
    256 floats; normalization, ReLU and row L2-norm run on-device.
"""

import sys
import numpy as np

if "/opt/trn_rl_repo" not in sys.path:
    sys.path.insert(0, "/opt/trn_rl_repo")

from contextlib import ExitStack

import concourse.bass as bass
import concourse.bacc as bacc
import concourse.mybir as mybir
import concourse.tile as tile
from concourse import masks
from concourse.bass_utils import run_bass_kernel_spmd

P = 128
D = 128
N_CORES = 8
N_NODES = 50000
TPC_FULL = 49  # tiles per core (8*49*128 = 50176 >= 50000)
GC = 7         # max chunks (x128 indices) per dma_gather instruction
NQ = 4         # SWDGE queues for parallel descriptor generation

f32 = mybir.dt.float32
f16 = mybir.dt.float16
i16 = mybir.dt.int16
i32 = mybir.dt.int32


def _ceil_div(a, b):
    return -(-a // b)


def host_prep(x, W_lin, W_mem, gamma, beta, edge_index, msg_recipients,
              n_nodes, n_cores, tpc):
    """Host-side index restructuring (no float math on the output path)."""
    B = tpc * P
    NPAD = n_cores * B
    T_ALL = n_cores * tpc
    IBASE = max(0, NPAD - 32768)

    src = np.asarray(edge_index[0], dtype=np.int64)
    dst = np.asarray(edge_index[1], dtype=np.int64)
    rec = np.asarray(msg_recipients, dtype=np.int64)
    loop = np.arange(n_nodes, dtype=np.int64)

    indeg = np.bincount(dst, minlength=NPAD).astype(np.int64)
    deg_full = indeg + 1  # self loop

    def bucket(tgt, who, lane_deg=None):
        """Group (who -> tgt) by destination tile; ragged per-tile lists."""
        key = tgt // P
        order = np.argsort(key, kind="stable")
        ks = key[order]
        counts = np.bincount(key, minlength=T_ALL)
        starts = np.zeros(T_ALL + 1, dtype=np.int64)
        np.cumsum(counts, out=starts[1:])
        whos = who[order]
        ids = (tgt[order] - ks * P)
        degs = lane_deg[order] if lane_deg is not None else None
        return counts, starts, whos, ids, degs

    e_cnt, e_st, e_who, e_ids_r, e_deg_r = bucket(dst, src, deg_full[src])
    m_cnt, m_st, m_who, m_ids_r, _ = bucket(rec, loop)

    ec = _ceil_div(np.maximum(e_cnt, 1), P)  # >=1 chunk to keep program uniform
    mc = _ceil_div(np.maximum(m_cnt, 1), P)

    # snake-deal tiles (sorted by edge chunks then mem chunks desc) to
    # (core, slot): tiles sharing a slot have near-equal chunk counts, so
    # the shared per-slot schedule wastes few pad lanes.
    order = np.argsort(-(ec * 1000 + mc).astype(np.int64) * 10**6
                       - e_cnt - m_cnt, kind="stable")
    perm = np.zeros((n_cores, tpc), dtype=np.int64)
    KEc = np.zeros(tpc, dtype=np.int64)
    KMc = np.zeros(tpc, dtype=np.int64)
    for s in range(tpc):
        grp = order[s * n_cores:(s + 1) * n_cores]
        perm[:, s] = grp
        KEc[s] = ec[grp].max()
        KMc[s] = mc[grp].max()

    KT = KEc + KMc
    # column offsets
    SO = np.zeros(tpc + 1, dtype=np.int64)   # cidx col offset (x8 per chunk)
    EO = np.zeros(tpc + 1, dtype=np.int64)   # eids col offset
    MO = np.zeros(tpc + 1, dtype=np.int64)   # mids col offset
    np.cumsum(KT * 8, out=SO[1:])
    np.cumsum(KEc, out=EO[1:])
    np.cumsum(KMc, out=MO[1:])
    COLS, ECOLS, MCOLS = int(SO[-1]), int(EO[-1]), int(MO[-1])

    # paired hi|lo fp16 table: 512B rows, one gather descriptor still moves
    # both halves; hi+lo reconstructs x to ~f32 precision on device.
    x32 = np.zeros((NPAD, D), dtype=np.float32)
    x32[:n_nodes] = np.asarray(x, dtype=np.float32)
    x_hi = x32.astype(np.float16)
    x_lo = (x32 - x_hi.astype(np.float32)).astype(np.float16)
    x_h = np.concatenate([x_hi, x_lo], axis=1)  # [NPAD, 2D]

    in_maps = []
    for c in range(n_cores):
        cidx = np.zeros((16, COLS), dtype=np.int16)
        eidc = np.full((P, ECOLS), -1.0, dtype=np.float32)
        edegc = np.ones((P, ECOLS), dtype=np.float32)
        midc = np.full((P, MCOLS), -1.0, dtype=np.float32)
        xloc = np.zeros((B, 2 * D), dtype=np.float16)
        degl = np.zeros((P, tpc), dtype=np.float32)
        for s in range(tpc):
            ti = perm[c, s]
            ke, km = int(KEc[s]), int(KMc[s])
            kt = ke + km
            lanes = np.zeros(kt * P, dtype=np.int16)
            n_e = int(e_cnt[ti])
            n_m = int(m_cnt[ti])
            le = e_who[e_st[ti]:e_st[ti] + n_e] - IBASE
            lm = m_who[m_st[ti]:m_st[ti] + n_m] - IBASE
            lanes[:n_e] = le.astype(np.int16)
            lanes[ke * P:ke * P + n_m] = lm.astype(np.int16)
            ids_e = np.full(ke * P, -1.0, dtype=np.float32)
            ids_e[:n_e] = e_ids_r[e_st[ti]:e_st[ti] + n_e]
            deg_e = np.ones(ke * P, dtype=np.float32)
            deg_e[:n_e] = e_deg_r[e_st[ti]:e_st[ti] + n_e]
            ids_m = np.full(km * P, -1.0, dtype=np.float32)
            ids_m[:n_m] = m_ids_r[m_st[ti]:m_st[ti] + n_m]

            # tail-strip guard: last lane of each <=GC-chunk window must be
            # >= 0 (the gather ucode drops trailing negative indices).
            E_LANES = ke * P
            for w0 in range(0, kt, GC):
                w1 = min(w0 + GC, kt)
                last = w1 * P - 1
                if lanes[last] >= 0:
                    continue
                lo = w0 * P if last < E_LANES else max(w0 * P, E_LANES)
                hi = min(w1 * P, E_LANES) if last < E_LANES else w1 * P
                cand = np.nonzero(lanes[lo:hi] >= 0)[0]
                if len(cand) == 0:
                    raise RuntimeError("no non-negative lane in window")
                j = lo + int(cand[-1])
                lanes[last], lanes[j] = lanes[j], lanes[last]
                if last < E_LANES:
                    ids_e[last], ids_e[j] = ids_e[j], ids_e[last]
                    deg_e[last], deg_e[j] = deg_e[j], deg_e[last]
                else:
                    ml, mj = last - E_LANES, j - E_LANES
                    ids_m[ml], ids_m[mj] = ids_m[mj], ids_m[ml]

            # wrap16 into cidx block: value (r, col8) = lanes[col8*16 + r]
            cidx[:, SO[s]:SO[s + 1]] = lanes.reshape(kt * 8, 16).T
            # ids/deg as [P, K] blocks: col k, row p = lane k*128+p
            eidc[:, EO[s]:EO[s + 1]] = ids_e.reshape(ke, P).T
            edegc[:, EO[s]:EO[s + 1]] = deg_e.reshape(ke, P).T
            midc[:, MO[s]:MO[s + 1]] = ids_m.reshape(km, P).T
            xloc[s * P:(s + 1) * P] = x_h[ti * P:(ti + 1) * P]
            degl[:, s] = indeg[ti * P:(ti + 1) * P].astype(np.float32)

        in_maps.append({
            "xh": x_h,
            "xloc": xloc,
            "wlin": np.asarray(W_lin, dtype=np.float32),
            "wmem": np.asarray(W_mem, dtype=np.float32),
            "gamma": np.asarray(gamma, dtype=np.float32).reshape(1, D),
            "beta": np.asarray(beta, dtype=np.float32).reshape(1, D),
            "degl": degl,
            "cidx": np.ascontiguousarray(np.tile(cidx, (8, 1))),
            "eids": eidc,
            "edegs": edegc,
            "mids": midc,
        })
    sched = (tuple(int(v) for v in KEc), tuple(int(v) for v in KMc))
    return in_maps, sched, perm


def build_program(n_cores, tpc, sched, n_real, sim_mode=False):
    """Trace the SPMD Bass/Tile program (identical across cores)."""
    KEc, KMc = sched
    B = tpc * P
    NPAD = n_cores * B
    IBASE = max(0, NPAD - 32768)
    KT = [KEc[s] + KMc[s] for s in range(tpc)]
    KEmax = max(KEc)
    KMmax = max(KMc)
    KTmax = max(KT)
    SO = np.zeros(tpc + 1, dtype=np.int64)
    EO = np.zeros(tpc + 1, dtype=np.int64)
    MO = np.zeros(tpc + 1, dtype=np.int64)
    np.cumsum([k * 8 for k in KT], out=SO[1:])
    np.cumsum(KEc, out=EO[1:])
    np.cumsum(KMc, out=MO[1:])
    COLS, ECOLS, MCOLS = int(SO[-1]), int(EO[-1]), int(MO[-1])
    inv_n = 1.0 / float(n_real)

    nc = bacc.Bacc("TRN2", target_bir_lowering=False, debug=False,
                   num_devices=n_cores, num_swdge_queues=NQ)
    xh_d = nc.dram_tensor("xh", [NPAD, 2 * D], f16, kind="ExternalInput")
    xloc_d = nc.dram_tensor("xloc", [B, 2 * D], f16, kind="ExternalInput")
    wlin_d = nc.dram_tensor("wlin", [D, D], f32, kind="ExternalInput")
    wmem_d = nc.dram_tensor("wmem", [D, D], f32, kind="ExternalInput")
    gamma_d = nc.dram_tensor("gamma", [1, D], f32, kind="ExternalInput")
    beta_d = nc.dram_tensor("beta", [1, D], f32, kind="ExternalInput")
    degl_d = nc.dram_tensor("degl", [P, tpc], f32, kind="ExternalInput")
    cidx_d = nc.dram_tensor("cidx", [P, COLS], i16, kind="ExternalInput")
    eids_d = nc.dram_tensor("eids", [P, ECOLS], f32, kind="ExternalInput")
    edegs_d = nc.dram_tensor("edegs", [P, ECOLS], f32, kind="ExternalInput")
    mids_d = nc.dram_tensor("mids", [P, MCOLS], f32, kind="ExternalInput")
    out_d = nc.dram_tensor("out", [B, D], f32, kind="ExternalOutput")

    with tile.TileContext(nc) as tc, ExitStack() as ctx:
        const = ctx.enter_context(tc.tile_pool(name="const", bufs=1))
        dram = ctx.enter_context(tc.tile_pool(name="dram", bufs=1, space="DRAM"))

        # ---- constants -------------------------------------------------
        ident_f32 = const.tile([P, P], f32)
        masks.make_identity(nc, ident_f32[:])
        ident_h = const.tile([P, P], f16)
        nc.vector.tensor_copy(ident_h[:], ident_f32[:])
        iota_i = const.tile([P, P], i16)
        nc.gpsimd.iota(iota_i[:], pattern=[[1, P]], base=0, channel_multiplier=0)
        iota_h = const.tile([P, P], f16)
        nc.vector.tensor_copy(iota_h[:], iota_i[:])
        ones_col = const.tile([P, 1], f32)
        nc.vector.memset(ones_col[:], 1.0)
        ones_row = const.tile([1, P], f32)
        nc.vector.memset(ones_row[:], 1.0)

        wlin_f = const.tile([D, D], f32)
        nc.sync.dma_start(wlin_f[:], wlin_d[:, :])
        wmem_f = const.tile([D, D], f32)
        nc.sync.dma_start(wmem_f[:], wmem_d[:, :])

        gamma_t = const.tile([1, D], f32)
        nc.sync.dma_start(gamma_t[:], gamma_d[:, :])
        beta_t = const.tile([1, D], f32)
        nc.sync.dma_start(beta_t[:], beta_d[:, :])

        cidx_t = const.tile([P, COLS], i16)
        nc.sync.dma_start(cidx_t[:], cidx_d[:, :])
        eids_f = const.tile([P, ECOLS], f32)
        nc.sync.dma_start(eids_f[:], eids_d[:, :])
        eids_t = const.tile([P, ECOLS], f16)
        nc.vector.tensor_copy(eids_t[:], eids_f[:])
        edegs_t = const.tile([P, ECOLS], f32)
        nc.sync.dma_start(edegs_t[:], edegs_d[:, :])
        esq_t = const.tile([P, ECOLS], f32)
        nc.scalar.activation(esq_t[:], edegs_t[:],
                             mybir.ActivationFunctionType.Sqrt)
        edinv_f = const.tile([P, ECOLS], f32)
        nc.vector.reciprocal(edinv_f[:], esq_t[:])
        edinv_t = const.tile([P, ECOLS], f16)
        nc.vector.tensor_copy(edinv_t[:], edinv_f[:])
        mids_f = const.tile([P, MCOLS], f32)
        nc.sync.dma_start(mids_f[:], mids_d[:, :])
        mids_t = const.tile([P, MCOLS], f16)
        nc.vector.tensor_copy(mids_t[:], mids_f[:])

        # local dinv / dinv^2 from integer in-degrees (self loop via bias)
        degl_t = const.tile([P, tpc], f32)
        nc.sync.dma_start(degl_t[:], degl_d[:, :])
        dsq = const.tile([P, tpc], f32)
        nc.scalar.activation(dsq[:], degl_t[:],
                             mybir.ActivationFunctionType.Sqrt, bias=1.0)
        dinv_l = const.tile([P, tpc], f32)
        nc.vector.reciprocal(dinv_l[:], dsq[:])
        dinv2_l = const.tile([P, tpc], f32)
        nc.vector.tensor_tensor(dinv2_l[:], dinv_l[:], dinv_l[:],
                                mybir.AluOpType.mult)

        agg = const.tile([P, tpc * P], f32)          # resident aggregate
        zeros_t = const.tile([P, D], f32)
        nc.vector.memset(zeros_t[:], 0.0)
        eps12 = const.tile([P, 1], f32)
        nc.vector.memset(eps12[:], 1e-12)

        # warm up the collective path early so the real AllReduce at the
        # end doesn't pay one-time setup; runs concurrently with phase 2.
        cc_win = dram.tile([1, 2 * D], f32)
        cc_wout = dram.tile([1, 2 * D], f32)
        if False:
            warm = const.tile([1, 2 * D], f32)
            nc.vector.memset(warm[:], 0.0)
            nc.sync.dma_start(cc_win[:], warm[:])
            nc.gpsimd.collective_compute(
                "AllReduce", mybir.AluOpType.add,
                replica_groups=[list(range(n_cores))],
                ins=[cc_win.opt()], outs=[cc_wout.opt()])

        # ---- phase 2: gather + aggregate per slot ----------------------
        xh_base = xh_d[IBASE:IBASE + P, :]
        rr = 0
        with tc.tile_pool(name="gat", bufs=4) as gat, \
             tc.tile_pool(name="xlp", bufs=3) as xlp, \
             tc.tile_pool(name="ohp", bufs=2) as ohp, \
             tc.tile_pool(name="ohw", bufs=2) as ohw, \
             tc.tile_pool(name="ohm", bufs=2) as ohmp, \
             tc.tile_pool(name="work", bufs=4) as work, \
             tc.tile_pool(name="psA", bufs=2, space="PSUM") as psAp, \
             tc.tile_pool(name="psO", bufs=2, space="PSUM") as psOp, \
             tc.tile_pool(name="psT", bufs=1, space="PSUM") as psTp, \
             tc.tile_pool(name="psR", bufs=1, space="PSUM") as psRp, \
             tc.tile_pool(name="psS", bufs=1, space="PSUM") as psSp, \
             tc.tile_pool(name="sq", bufs=2) as sqp:

            s1 = psSp.tile([1, D], f32, tag="s1")
            s2 = psSp.tile([1, D], f32, tag="s2")
            for s in range(tpc):
                ke, km = KEc[s], KMc[s]
                kt = ke + km
                so, eo, mo = int(SO[s]), int(EO[s]), int(MO[s])

                gt = gat.tile([P, KTmax, 2 * D], f16, tag="gt")
                for k0 in range(0, kt, GC):
                    k1 = min(k0 + GC, kt)
                    nidx = (k1 - k0) * P
                    nc.gpsimd.dma_gather(
                        gt[:, k0:k1, :], xh_base,
                        cidx_t[:, so + k0 * 8: so + k1 * 8],
                        nidx, nidx, 2 * D, queue_num=rr % NQ)
                    rr += 1

                xt = xlp.tile([P, 2 * D], f16, tag="xt")
                nc.sync.dma_start(xt[:], xloc_d[s * P:(s + 1) * P, :])

                # -- wide one-hots for this slot ---------------------------
                ohe = ohp.tile([P, KEmax, P], f16, tag="ohe")
                nc.vector.tensor_tensor(
                    ohe[:, 0:ke, :],
                    iota_h[:, None, :].broadcast_to([P, ke, P]),
                    eids_t[:, eo:eo + ke, None].broadcast_to([P, ke, P]),
                    mybir.AluOpType.is_equal)
                ohew = ohw.tile([P, KEmax, P], f16, tag="ohew")
                nc.vector.tensor_tensor(
                    ohew[:, 0:ke, :], ohe[:, 0:ke, :],
                    edinv_t[:, eo:eo + ke, None].broadcast_to([P, ke, P]),
                    mybir.AluOpType.mult)
                ohm = ohmp.tile([P, KMmax, P], f16, tag="ohm")
                nc.vector.tensor_tensor(
                    ohm[:, 0:km, :],
                    iota_h[:, None, :].broadcast_to([P, km, P]),
                    mids_t[:, mo:mo + km, None].broadcast_to([P, km, P]),
                    mybir.AluOpType.is_equal)

                # -- GCN edges into psA (hi and lo halves) -----------------
                psA = psAp.tile([P, D], f32, tag="psA")
                for k in range(ke):
                    nc.tensor.matmul(psA[:], ohew[:, k, :], gt[:, k, 0:D],
                                     start=(k == 0), stop=False)
                    nc.tensor.matmul(psA[:], ohew[:, k, :], gt[:, k, D:2 * D],
                                     start=False, stop=(k == ke - 1))

                # a2 = dinv * psA + dinv^2 * (x_hi + x_lo)   (f32)
                a1 = work.tile([P, D], f32, tag="a1")
                nc.scalar.activation(a1[:], psA[:],
                                     mybir.ActivationFunctionType.Copy,
                                     scale=dinv_l[:, s:s + 1])
                xs = work.tile([P, D], f32, tag="xs")
                nc.vector.tensor_tensor(xs[:], xt[:, 0:D], xt[:, D:2 * D],
                                        mybir.AluOpType.add)
                xl = work.tile([P, D], f32, tag="xl")
                nc.scalar.activation(xl[:], xs[:],
                                     mybir.ActivationFunctionType.Copy,
                                     scale=dinv2_l[:, s:s + 1])
                a2 = work.tile([P, D], f32, tag="a2")
                nc.vector.tensor_tensor(a2[:], a1[:], xl[:],
                                        mybir.AluOpType.add)
                psT2 = psTp.tile([P, P], f32, tag="psT")
                nc.tensor.transpose(psT2[:], a2[:], ident_f32[:])
                aT = work.tile([P, P], f32, tag="aT")
                nc.scalar.copy(aT[:], psT2[:])

                # -- memory messages into psO ------------------------------
                psO = psOp.tile([P, D], f32, tag="psO")
                for k in range(km):
                    xg = work.tile([P, D], f32, tag="xg")
                    nc.vector.tensor_tensor(xg[:], gt[:, ke + k, 0:D],
                                            gt[:, ke + k, D:2 * D],
                                            mybir.AluOpType.add)
                    psT = psTp.tile([P, P], f32, tag="psT")
                    nc.tensor.transpose(psT[:], xg[:], ident_f32[:])
                    xT = work.tile([P, P], f32, tag="xT")
                    nc.scalar.copy(xT[:], psT[:])
                    psR = psRp.tile([P, D], f32, tag="psR")
                    nc.tensor.matmul(psR[:], xT[:], wmem_f[:],
                                     start=True, stop=True)
                    rv = work.tile([P, D], f16, tag="rv")
                    nc.scalar.activation(rv[:], psR[:],
                                         mybir.ActivationFunctionType.Lrelu,
                                         alpha=0.01)
                    nc.tensor.matmul(psO[:], ohm[:, k, :], rv[:],
                                     start=(k == 0), stop=False)

                # agg_gcn = a2 @ W_lin (f32) -> psO accumulate, then drain
                nc.tensor.matmul(psO[:], aT[:], wlin_f[:],
                                 start=False, stop=True)
                sl = agg[:, s * P:(s + 1) * P]
                nc.scalar.copy(sl, psO[:])

                # inline BN stats for this slot (overlaps later gathers)
                sq = sqp.tile([P, P], f32, tag="sqt")
                nc.vector.tensor_tensor(sq[:], sl, sl, mybir.AluOpType.mult)
                nc.tensor.matmul(s1[:], ones_col[:], sl,
                                 start=(s == 0), stop=(s == tpc - 1))
                nc.tensor.matmul(s2[:], ones_col[:], sq[:],
                                 start=(s == 0), stop=(s == tpc - 1))

            # drain accumulated stats while pools are still alive
            stats = const.tile([1, 2 * D], f32)
            nc.vector.tensor_copy(stats[:, 0:D], s1[:])
            nc.vector.tensor_copy(stats[:, D:2 * D], s2[:])

        # ---- phase 3: AllReduce of BN stats -----------------------------

        cc_in = dram.tile([1, 2 * D], f32)
        cc_out = dram.tile([1, 2 * D], f32)
        nc.sync.dma_start(cc_in[:], stats[:])
        if sim_mode:
            nc.gpsimd.dma_start(cc_out[:], cc_in[:])
        else:
            nc.gpsimd.collective_compute(
                "AllReduce", mybir.AluOpType.add,
                replica_groups=[list(range(n_cores))],
                ins=[cc_in.opt()], outs=[cc_out.opt()])
        gstats = const.tile([1, 2 * D], f32)
        nc.sync.dma_start(gstats[:], cc_out[:])

        mu = const.tile([1, D], f32)
        nc.vector.tensor_scalar(out=mu[:], in0=gstats[:, 0:D], scalar1=inv_n,
                                scalar2=None, op0=mybir.AluOpType.mult)
        ex2 = const.tile([1, D], f32)
        nc.vector.tensor_scalar(out=ex2[:], in0=gstats[:, D:2 * D], scalar1=inv_n,
                                scalar2=None, op0=mybir.AluOpType.mult)
        musq = const.tile([1, D], f32)
        nc.vector.tensor_tensor(musq[:], mu[:], mu[:], mybir.AluOpType.mult)
        var = const.tile([1, D], f32)
        nc.vector.tensor_tensor(var[:], ex2[:], musq[:], mybir.AluOpType.subtract)
        eps = const.tile([1, 1], f32)
        nc.vector.memset(eps[:], 1e-5)
        std = const.tile([1, D], f32)
        nc.scalar.activation(std[:], var[:],
                             mybir.ActivationFunctionType.Sqrt, bias=eps[:])
        istd = const.tile([1, D], f32)
        nc.vector.reciprocal(istd[:], std[:])
        arow = const.tile([1, D], f32)
        nc.vector.tensor_tensor(arow[:], gamma_t[:], istd[:], mybir.AluOpType.mult)
        tmp = const.tile([1, D], f32)
        nc.vector.tensor_tensor(tmp[:], mu[:], arow[:], mybir.AluOpType.mult)
        brow = const.tile([1, D], f32)
        nc.vector.tensor_tensor(brow[:], beta_t[:], tmp[:], mybir.AluOpType.subtract)

        ab = const.tile([P, D], f32)
        bb = const.tile([P, D], f32)
        with tc.tile_pool(name="psB", bufs=1, space="PSUM") as psBp:
            pa = psBp.tile([P, D], f32, tag="pa")
            nc.tensor.matmul(pa[:], ones_row[:], arow[:], start=True, stop=True)
            nc.vector.tensor_copy(ab[:], pa[:])
            pb = psBp.tile([P, D], f32, tag="pb")
            nc.tensor.matmul(pb[:], ones_row[:], brow[:], start=True, stop=True)
            nc.vector.tensor_copy(bb[:], pb[:])

        # ---- phase 4: normalize + relu + L2 ----------------------------
        # spread across GpSimd (idle after gathers) / DVE / Scalar
        with tc.tile_pool(name="fin", bufs=4) as fin:
            for t in range(tpc):
                sl = agg[:, t * P:(t + 1) * P]
                y1 = fin.tile([P, D], f32, tag="y1")
                nc.vector.tensor_tensor(y1[:], sl, ab[:], mybir.AluOpType.mult)
                y2 = fin.tile([P, D], f32, tag="y2")
                nc.vector.tensor_tensor(y2[:], y1[:], bb[:], mybir.AluOpType.add)
                y3 = fin.tile([P, D], f32, tag="y3")
                nc.vector.tensor_tensor(y3[:], y2[:], zeros_t[:],
                                        mybir.AluOpType.max)
                sqd = fin.tile([P, D], f32, tag="sqd")
                ss = fin.tile([P, 1], f32, tag="ss")
                nc.scalar.activation(sqd[:], y3[:],
                                     mybir.ActivationFunctionType.Square,
                                     accum_out=ss[:])
                nrm = fin.tile([P, 1], f32, tag="nrm")
                nc.scalar.activation(nrm[:], ss[:],
                                     mybir.ActivationFunctionType.Sqrt)
                nc.vector.tensor_tensor(nrm[:], nrm[:], eps12[:],
                                        mybir.AluOpType.max)
                rn = fin.tile([P, 1], f32)
                nc.vector.reciprocal(rn[:], nrm[:])
                yf = fin.tile([P, D], f32, tag="yf")
                nc.scalar.activation(yf[:], y3[:],
                                     mybir.ActivationFunctionType.Copy,
                                     scale=rn[:])
                nc.sync.dma_start(out_d[t * P:(t + 1) * P, :], yf[:])

    nc.compile()
    return nc


_CACHE = {}


def _run(x, W_lin, W_mem, gamma, beta, edge_index, msg_recipients,
         n_nodes, n_cores, tpc, trace=False):
    in_maps, sched, perm = host_prep(x, W_lin, W_mem, gamma, beta, edge_index,
                                     msg_recipients, n_nodes, n_cores, tpc)
    key = (n_cores, tpc, sched, n_nodes)
    if key not in _CACHE:
        _CACHE[key] = build_program(n_cores, tpc, sched, n_nodes)
    nc = _CACHE[key]
    res = run_bass_kernel_spmd(nc, in_maps, list(range(n_cores)), trace=trace)
    B = tpc * P
    out = np.zeros((n_cores * B, D), dtype=np.float32)
    for c in range(n_cores):
        oc = res.results[c]["out"]
        for s in range(tpc):
            ti = perm[c, s]
            out[ti * P:(ti + 1) * P] = oc[s * P:(s + 1) * P]
    return out[:n_nodes], res


def kernel(**inputs):
    out, _ = _run(
        inputs["x"], inputs["W_lin"], inputs["W_mem"], inputs["gamma"],
        inputs["beta"], inputs["edge_index"], inputs["msg_recipients"],
        N_NODES, N_CORES, TPC_FULL)
    return np.ascontiguousarray(out, dtype=np.float32)
